# revision 2
# baseline (speedup 1.0000x reference)
"""Single-head causal attention (B=8, T=4096, EMB=1024, HEAD=64) on 8 trn2 cores.

Strategy: data-parallel over batch, one batch element per NeuronCore.

Per core (all matmuls in bf16, fp32 PSUM accumulation):
  1. QKV projection from host-pretransposed xT [1024, 4096]:
       KQ^T [128, 4096]  (rows 0:64 = K^T, 64:128 = Q^T), via W[:, 0:128] stationary
       V    [4096, 64]   natural layout, via xT-chunk stationary x Wv moving
  2. Q^T copied to partitions 0:63 (sbuf->sbuf DMA) so scores matmuls can
     contract over d=64 on partitions 0:63.
  3. Scores S^T[s, t] = K @ Q^T per (s-chunk 128, t-tile 512), PSUM fp32.
     No max-subtraction: scores ~ N(0, 0.41), exp is safe in fp32.
  4. exp via ScalarE directly from PSUM (scale=1/8 folded in), bf16 out.
     Causal: only tiles with t >= s computed; diagonal 128x128 block masked
     by a 0/1 multiply after exp.
  5. PV: P^T tile [128s, 128t] stationary, V-with-ones-column [128, 65] moving
     -> accumulates [O_unnorm | rowsum] in natural [t, 65] layout.
  6. out = O_unnorm * reciprocal(rowsum) (per-partition scalar), DMA out fp32.
"""

from contextlib import ExitStack

import numpy as np
import ml_dtypes

B, T, EMB, HEAD = 8, 4096, 1024, 64
KCH = EMB // 128          # 8 contraction chunks
NTT = T // 512            # 8 t-tiles of 512
NTS = T // 128            # 32 t-subtiles / s-chunks of 128
BF16 = ml_dtypes.bfloat16

_CACHE = {}


def _build_program():
    import concourse.bacc as bacc
    import concourse.tile as tile
    from concourse import mybir
    from concourse.masks import make_identity

    fp32 = mybir.dt.float32
    bf16 = mybir.dt.bfloat16
    EXP = mybir.ActivationFunctionType.Exp

    nc = bacc.Bacc("TRN2", target_bir_lowering=False, debug=False)
    xt_ap = nc.dram_tensor("xt", [EMB, T], bf16, kind="ExternalInput").ap()
    w_ap = nc.dram_tensor("w", [EMB, 192], bf16, kind="ExternalInput").ap()
    mask_ap = nc.dram_tensor("mask", [128, 128], bf16, kind="ExternalInput").ap()
    o_ap = nc.dram_tensor("o", [T, HEAD], fp32, kind="ExternalOutput").ap()

    with tile.TileContext(nc) as tc:
        with (
            tc.tile_pool(name="consts", bufs=1) as consts,
            tc.tile_pool(name="outs", bufs=4) as outs,
        ):
            # ---------- constants ----------
            w_sb = consts.tile([128, KCH, 192], bf16, tag="w")
            for k in range(KCH):
                nc.sync.dma_start(out=w_sb[:, k, :], in_=w_ap[k * 128:(k + 1) * 128, :])
            mask_sb = consts.tile([128, 128], bf16, tag="mask")
            nc.sync.dma_start(out=mask_sb, in_=mask_ap)
            ident_sb = consts.tile([128, 128], fp32, tag="ident")
            make_identity(nc, ident_sb)
            # V with ones column: [128, 65] per s-chunk; col 64 preset to 1.0
            vt_sb = consts.tile([128, NTS * 65], bf16, tag="vt")
            nc.gpsimd.memset(vt_sb, 1.0)

            kq_sb = consts.tile([128, T], bf16, tag="kq")
            qk_sb = consts.tile([128, T], bf16, tag="qk")

            # ---------- phase 1: load x, project (k-outer: PE starts as
            # each x chunk lands; KQ in two half-T passes + V share 8 banks)
            with (
                tc.tile_pool(name="xp", bufs=1) as xp,
                tc.tile_pool(name="ps_kq", bufs=1, space="PSUM") as ps_kq,
                tc.tile_pool(name="ps_v", bufs=1, space="PSUM") as ps_v,
            ):
                xt_sb = xp.tile([128, KCH, T], bf16, tag="xt")
                for k in range(KCH):
                    nc.sync.dma_start(
                        out=xt_sb[:, k, :], in_=xt_ap[k * 128:(k + 1) * 128, :]
                    )

                pkq = []
                for j in range(4):
                    pkq_j = ps_kq.tile([128, 512], fp32, tag=f"kq{j}")
                    pkq.append(pkq_j)
                pv = []
                for g in range(4):
                    pv_g = ps_v.tile([128, 512], fp32, tag=f"v{g}")
                    pv.append(pv_g)

                # pass 1 over k: KQ t-tiles 0..3 and all V accumulators
                for k in range(KCH):
                    for j in range(4):
                        nc.tensor.matmul(
                            pkq[j],
                            w_sb[:, k, 0:128],
                            xt_sb[:, k, j * 512:(j + 1) * 512],
                            start=(k == 0),
                            stop=(k == KCH - 1),
                            skip_group_check=True,
                        )
                    for i in range(NTS):
                        # start=True clears the WHOLE bank's has_written bits,
                        # so only the first accumulator in each bank may issue
                        # it; the rest overwrite-on-first-write via per-element
                        # has_written.
                        nc.tensor.matmul(
                            pv[i // 8][:, (i % 8) * 64:(i % 8 + 1) * 64],
                            xt_sb[:, k, i * 128:(i + 1) * 128],
                            w_sb[:, k, 128:192],
                            start=(k == 0 and i % 8 == 0),
                            stop=(k == KCH - 1),
                            skip_group_check=True,
                        )
                for j in range(4):
                    nc.vector.tensor_copy(kq_sb[:, j * 512:(j + 1) * 512], pkq[j])
                for i in range(NTS):
                    nc.vector.tensor_copy(
                        vt_sb[:, i * 65:i * 65 + 64],
                        pv[i // 8][:, (i % 8) * 64:(i % 8 + 1) * 64],
                    )
                # pass 2 over k (x fully resident): KQ t-tiles 4..7
                pkq2 = []
                for j in range(4):
                    pkq2_j = ps_kq.tile([128, 512], fp32, tag=f"kq{j}")
                    pkq2.append(pkq2_j)
                for k in range(KCH):
                    for j in range(4):
                        nc.tensor.matmul(
                            pkq2[j],
                            w_sb[:, k, 0:128],
                            xt_sb[:, k, (j + 4) * 512:(j + 5) * 512],
                            start=(k == 0),
                            stop=(k == KCH - 1),
                            skip_group_check=True,
                        )
                for j in range(4):
                    nc.vector.tensor_copy(
                        kq_sb[:, (j + 4) * 512:(j + 5) * 512], pkq2[j]
                    )
                # Q^T to low partitions for scores moving operand
                nc.sync.dma_start(out=qk_sb[0:64, :], in_=kq_sb[64:128, :])

            # ---------- phase 2: attention ----------
            phase2 = ExitStack()
            ptp = phase2.enter_context(tc.tile_pool(name="pt", bufs=1))
            ps_s = phase2.enter_context(tc.tile_pool(name="ps_s", bufs=2, space="PSUM"))
            ps_o = phase2.enter_context(tc.tile_pool(name="ps_o", bufs=1, space="PSUM"))
            pt = []
            for a in range(NTS):
                pt_a = ptp.tile([128, T - 128 * a], bf16, tag=f"pt{a}")
                pt.append(pt_a)

            def score_groups(a):
                """[(jstart, gsize), ...] groups of <=3 t-tiles for s-chunk a."""
                j0 = a // 4
                groups = []
                j = j0
                while j < NTT:
                    g = min(3, NTT - j)
                    groups.append((j, g))
                    j += g
                return groups

            def emit_scores(a):
                tiles = []
                for (jstart, g) in score_groups(a):
                    psg = ps_s.tile([128, 512 * g], fp32, tag="sg")
                    for idx in range(g):
                        j = jstart + idx
                        nc.tensor.matmul(
                            psg[:, idx * 512:(idx + 1) * 512],
                            kq_sb[0:64, a * 128:(a + 1) * 128],
                            qk_sb[0:64, j * 512:(j + 1) * 512],
                            start=True,
                            stop=True,
                        )
                    tiles.append((jstart, g, psg))
                return tiles

            def emit_exp(a, tiles):
                for (jstart, g, psg) in tiles:
                    skip = max(0, 128 * a - 512 * jstart)
                    out_lo = 512 * jstart + skip - 128 * a
                    out_hi = 512 * (jstart + g) - 128 * a
                    nc.scalar.activation(
                        pt[a][:, out_lo:out_hi],
                        psg[:, skip:512 * g],
                        EXP,
                        scale=0.125,
                    )
                # mask the diagonal 128x128 block (zero where s > t)
                nc.vector.tensor_mul(pt[a][:, 0:128], pt[a][:, 0:128], mask_sb)

            def emit_pv(i):
                po = ps_o.tile([128, 65], fp32, tag="o")
                for aa in range(i + 1):
                    nc.tensor.matmul(
                        po,
                        pt[aa][:, 128 * (i - aa):128 * (i - aa) + 128],
                        vt_sb[:, aa * 65:(aa + 1) * 65],
                        start=(aa == 0),
                        stop=(aa == i),
                    )
                dr = outs.tile([128, 1], fp32, tag="dr")
                nc.vector.reciprocal(dr, po[:, 64:65])
                o_sb = outs.tile([128, 64], fp32, tag="o_sb")
                nc.vector.tensor_scalar_mul(o_sb, po[:, 0:64], dr)
                nc.sync.dma_start(out=o_ap[i * 128:(i + 1) * 128, :], in_=o_sb)

            # software-pipelined: while ACT(a) drains, PE runs S(a+1); PV for
            # t-tile j fires once its last needed chunk (4j+3) is exp'd.
            tiles = emit_scores(0)
            for a in range(NTS):
                emit_exp(a, tiles)
                if a + 1 < NTS:
                    tiles = emit_scores(a + 1)
                if a >= 1:
                    emit_pv(a - 1)
            emit_pv(NTS - 1)
            phase2.close()

    nc.compile()
    return nc


def _get_nc():
    if "nc" not in _CACHE:
        _CACHE["nc"] = _build_program()
    return _CACHE["nc"]


def _make_in_maps(x, W):
    x = np.asarray(x, dtype=np.float32)
    W = np.asarray(W, dtype=np.float32)
    assert x.shape == (B, T, EMB) and W.shape == (EMB, 3 * HEAD)

    xt = np.ascontiguousarray(x.transpose(0, 2, 1)).astype(BF16)  # [B, EMB, T]
    w16 = W.astype(BF16)
    mask = np.triu(np.ones((128, 128), np.float32)).astype(BF16)
    return [{"xt": xt[b], "w": w16, "mask": mask} for b in range(B)]


def kernel(x, W):
    from concourse.bass_utils import run_bass_kernel_spmd

    nc = _get_nc()
    in_maps = _make_in_maps(x, W)
    res = run_bass_kernel_spmd(nc, in_maps, list(range(B)))
    return np.stack([res.results[b]["o"] for b in range(B)]).astype(np.float32)



# revision 18
# speedup vs baseline: 1.1477x; 1.1477x over previous
"""Single-head causal attention (B=8, T=4096, EMB=1024, HEAD=64) on 8 trn2 cores.

Strategy: data-parallel over batch, one batch element per NeuronCore.

v2: t-tile-streamed pipeline (8 tiles of 512), designed so the ACT engine's
exp stream starts ~5us in and every engine stays continuously busy (HAM warm):

Per t-tile j (all matmuls bf16, fp32 PSUM):
  1. DMA x tile j (one contiguous 1MB transfer, host layout [j][p][k][c]).
  2. KQ^T tile: 8 matmuls, w_kq chunk stationary, xt moving 512-wide.
     K^T -> ks_sb low partitions, Q^T -> qs_sb high partitions (DVE), then
     sbuf-sbuf DMA duplicates each onto the other partition half so score
     matmuls can be packed two-per-PE-pass via tile_position row tiling
     (contraction is only d=64, so two independent 64-row matmuls share
     the 128x128 array).
  3. V^T tile via w_v stationary (64-wide) with 2x col tiling (two 256-col
     half-tiles concurrently), then 4 PE transposes -> V natural chunks
     [128s, 64d] stored next to a preset ones column (for the softmax
     denominator).
  4. Scores S^T chunk (a) = K chunk @ Q^T tile, row-tiled in (lo,hi) pairs.
     exp on ACT in 3-chunk [128,1536] groups straight out of PSUM (scale
     1/8 folded in), bf16 out to pt. Sub-diagonal garbage zeroed (GpSimd)
     and the diagonal 128x128 block masked (DVE).
  5. PV transposed: out^T[65, 512] += [V_aa | ones]^T @ P^T[aa] per s-chunk
     aa <= 4j+3; V stationary is only 65 columns so LDWEIGHTS hides under
     the 512-wide moving pass (the v1 kernel lost ~60us to per-matmul
     128-column weight loads here). Runs one exp-group behind scores.
  6. out^T tile -> SBUF fp32 -> DRAM [65, T]. Host divides by the Z row
     and transposes (no device normalization on the critical path).
"""

import os

import numpy as np
import ml_dtypes

B, T, EMB, HEAD = 8, 4096, 1024, 64
KCH = EMB // 128           # 8 contraction chunks
NTT = T // 512             # 8 t-tiles
NTS = T // 128             # 32 s-chunks
BF16 = ml_dtypes.bfloat16

# feature flags (bisect aids; defaults = full-speed configuration)
# NOTE: col tiling (tile_position=(0, 64)) crashes trn2 hw — never use it.
SC_PAIR = os.environ.get("BASS_SC_PAIR", "1") != "0"    # row-tiled score pairs
VNAT = os.environ.get("BASS_VNAT", "transpose")          # "transpose" | "direct"
# NOTE: is_transpose at base partition 64 also crashes hw; keep transposes
# on partitions 0:64 (TR_PAIR=0).
TR_PAIR = os.environ.get("BASS_TR_PAIR", "0") != "0"

_CACHE = {}


def _build_program():
    import concourse.bacc as bacc
    import concourse.tile as tile
    from concourse import mybir
    from concourse.masks import make_identity

    fp32 = mybir.dt.float32
    bf16 = mybir.dt.bfloat16
    EXP = mybir.ActivationFunctionType.Exp

    nc = bacc.Bacc("TRN2", target_bir_lowering=False, debug=False)
    xj_ap = nc.dram_tensor("xj", [NTT, 128, KCH, 512], bf16, kind="ExternalInput").ap()
    # per k-chunk 256 cols: [Wk | Wq] then [Wv | Wv] (dup -> V^T lands on both
    # partition halves so the V transposes can run as row-tiled lo/hi pairs)
    w_ap = nc.dram_tensor("w", [128, KCH * 256], bf16, kind="ExternalInput").ap()
    mask_ap = nc.dram_tensor("mask", [128, 128], bf16, kind="ExternalInput").ap()
    o_ap = nc.dram_tensor("o", [65, T], fp32, kind="ExternalOutput").ap()

    with tile.TileContext(nc) as tc:
        with (
            tc.tile_pool(name="consts", bufs=1) as consts,
            tc.tile_pool(name="ps_scr", bufs=1, space="PSUM") as ps_scr,
            tc.tile_pool(name="ps_sc", bufs=2, space="PSUM") as ps_sc,
            tc.tile_pool(name="ps_pv", bufs=1, space="PSUM") as ps_pv,
        ):
            # ---------- constants ----------
            w_sb = consts.tile([128, KCH * 256], bf16, tag="w")
            nc.sync.dma_start(out=w_sb, in_=w_ap)
            mask_sb = consts.tile([128, 128], bf16, tag="mask")
            nc.sync.dma_start(out=mask_sb, in_=mask_ap)

            xt_sb = consts.tile([128, NTT, KCH, 512], bf16, tag="xt")
            for j in range(NTT):
                nc.sync.dma_start(out=xt_sb[:, j], in_=xj_ap[j])

            # identity (fp32) on both partition halves, for PE transposes
            ident_sb = consts.tile([128, 64], fp32, tag="ident")
            make_identity(nc, ident_sb[0:64, :])
            nc.sync.dma_start(out=ident_sb[64:128, :], in_=ident_sb[0:64, :])

            # V natural chunks [128s, 64d | ones] per s-chunk
            vt_sb = consts.tile([128, NTS * 65], bf16, tag="vt")
            nc.gpsimd.memset(vt_sb, 1.0)

            ks_sb = consts.tile([128, T], bf16, tag="ks")   # K^T on both halves
            qs_sb = consts.tile([128, T], bf16, tag="qs")   # Q^T on both halves
            vts_sb = consts.tile([128, NTT * 512], fp32, tag="vts")  # V^T (dup)
            pt_sb = consts.tile([128, NTS * 512], bf16, tag="pt")    # P^T chunks
            ot_sb = consts.tile([65, T], fp32, tag="ot")             # out^T

            for j in range(NTT):
                jsl = slice(j * 512, (j + 1) * 512)

                # ---- KQ^T tile ----
                scr = ps_scr.tile([128, 512], fp32, tag="scr")
                for k in range(KCH):
                    nc.tensor.matmul(
                        scr,
                        w_sb[:, k * 256:k * 256 + 128],
                        xt_sb[:, j, k],
                        start=(k == 0),
                        stop=(k == KCH - 1),
                        skip_group_check=True,
                    )
                nc.vector.tensor_copy(ks_sb[0:64, jsl], scr[0:64, :])
                nc.vector.tensor_copy(qs_sb[64:128, jsl], scr[64:128, :])
                nc.sync.dma_start(out=ks_sb[64:128, jsl], in_=ks_sb[0:64, jsl])
                nc.sync.dma_start(out=qs_sb[0:64, jsl], in_=qs_sb[64:128, jsl])

                if VNAT == "direct":
                    # baseline-style: xt chunk stationary, w_v moving
                    for k in range(KCH):
                        for q in range(4):
                            nc.tensor.matmul(
                                scr[:, 256 + q * 64:256 + q * 64 + 64],
                                xt_sb[:, j, k, q * 128:(q + 1) * 128],
                                w_sb[:, k * 256 + 128:k * 256 + 192],
                                start=(k == 0 and q == 0),
                                stop=(k == KCH - 1),
                                skip_group_check=True,
                            )
                    for q in range(4):
                        i = 4 * j + q
                        nc.vector.tensor_copy(
                            vt_sb[:, i * 65:i * 65 + 64],
                            scr[:, 256 + q * 64:256 + q * 64 + 64],
                        )
                else:
                    # ---- V^T tile, duplicated on both partition halves ----
                    for k in range(KCH):
                        nc.tensor.matmul(
                            scr,
                            w_sb[:, k * 256 + 128:k * 256 + 256],
                            xt_sb[:, j, k],
                            start=(k == 0),
                            stop=(k == KCH - 1),
                            skip_group_check=True,
                        )
                    nc.vector.tensor_copy(
                        vts_sb[:, j * 512:(j + 1) * 512], scr
                    )

                    # ---- V natural via PE transposes (lo/hi row-tiled pairs)
                    for q in range(4):
                        half = 64 if (TR_PAIR and q % 2 == 1) else 0
                        src = vts_sb[half:half + 64,
                                     j * 512 + q * 128:j * 512 + q * 128 + 128]
                        nc.tensor.matmul(
                            scr[:, 256 + q * 64:256 + q * 64 + 64],
                            src,
                            ident_sb[half:half + 64, :],
                            is_transpose=True,
                            start=(q == 0),
                            stop=(q == 3),
                            skip_group_check=True,
                        )
                    for q in range(4):
                        i = 4 * j + q
                        nc.vector.tensor_copy(
                            vt_sb[:, i * 65:i * 65 + 64],
                            scr[:, 256 + q * 64:256 + q * 64 + 64],
                        )

                # ---- scores + exp + PV, pipelined by one exp-group ----
                po = ps_pv.tile([65, 512], fp32, tag="pv")
                nchunk = 4 * j + 4
                groups = [list(range(g, min(g + 3, nchunk))) for g in range(0, nchunk, 3)]

                def emit_pv(chunks, j=j, po=po, nchunk=nchunk):
                    for aa in chunks:
                        nc.tensor.matmul(
                            po,
                            vt_sb[:, aa * 65:(aa + 1) * 65],
                            pt_sb[:, aa * 512:(aa + 1) * 512],
                            start=(aa == 0),
                            stop=(aa == nchunk - 1),
                            skip_group_check=True,
                        )

                for gi, chunks in enumerate(groups):
                    sc = ps_sc.tile([128, 1536], fp32, tag="sc")
                    for ci, a in enumerate(chunks):
                        half = 64 if (SC_PAIR and a % 2 == 1) else 0
                        nc.tensor.matmul(
                            sc[:, ci * 512:(ci + 1) * 512],
                            ks_sb[half:half + 64, a * 128:(a + 1) * 128],
                            qs_sb[half:half + 64, jsl],
                            start=True,
                            stop=True,
                            skip_group_check=True,
                        )
                    cnt = len(chunks)
                    a0 = chunks[0]
                    nc.scalar.activation(
                        pt_sb[:, a0 * 512:(a0 + cnt) * 512],
                        sc[:, 0:cnt * 512],
                        EXP,
                        scale=0.125,
                    )
                    for a in chunks:
                        if a >= 4 * j:
                            sub = a - 4 * j
                            if sub > 0:
                                nc.gpsimd.memset(
                                    pt_sb[:, a * 512:a * 512 + 128 * sub], 0.0
                                )
                            dsl = slice(a * 512 + 128 * sub, a * 512 + 128 * sub + 128)
                            nc.vector.tensor_mul(pt_sb[:, dsl], pt_sb[:, dsl], mask_sb)
                    if gi >= 1:
                        emit_pv(groups[gi - 1])
                emit_pv(groups[-1])

                nc.vector.tensor_copy(ot_sb[:, jsl], po)
                nc.sync.dma_start(out=o_ap[:, jsl], in_=ot_sb[:, jsl])

    nc.compile()
    return nc


def _get_nc():
    if "nc" not in _CACHE:
        _CACHE["nc"] = _build_program()
    return _CACHE["nc"]


def _make_in_maps(x, W):
    x = np.asarray(x, dtype=np.float32)
    W = np.asarray(W, dtype=np.float32)
    assert x.shape == (B, T, EMB) and W.shape == (EMB, 3 * HEAD)

    wc = W.astype(BF16).reshape(KCH, 128, 192)  # [k, p, c]: [Wk | Wq | Wv]
    wj = (
        np.concatenate([wc, wc[:, :, 128:192]], axis=2)  # append Wv dup
        .transpose(1, 0, 2)
        .reshape(128, KCH * 256)
        .copy()
    )
    mask = np.triu(np.ones((128, 128), np.float32)).astype(BF16)
    in_maps = []
    for b in range(B):
        xb = x[b].astype(BF16)  # [T, EMB]
        # xj[j, p, k, c] = x[512j + c, 128k + p]
        xj = np.ascontiguousarray(
            xb.reshape(NTT, 512, KCH, 128).transpose(0, 3, 2, 1)
        )
        in_maps.append({"xj": xj, "w": wj, "mask": mask})
    return in_maps


def _postprocess(o):
    # o: [65, T] fp32 -> [T, HEAD] normalized
    return (o[0:HEAD, :] / o[HEAD:HEAD + 1, :]).T


def kernel(x, W):
    from concourse.bass_utils import run_bass_kernel_spmd

    nc = _get_nc()
    in_maps = _make_in_maps(x, W)
    res = run_bass_kernel_spmd(nc, in_maps, list(range(B)))
    return np.stack(
        [_postprocess(res.results[b]["o"]) for b in range(B)]
    ).astype(np.float32)
